# revision 1
# baseline (speedup 1.0000x reference)
"""Trainium2 Bass kernel for per-pixel MDN head (nn_MDN_38946763440904).

Reference computation (per pixel, channels-first):
  h      = relu(W1 @ x5 + b1)        # 5  -> 32
  h      = relu(W2 @ h + b2)         # 32 -> 32
  latent = relu(W3 @ h + b3)         # 32 -> 32
  for c in (r, g, b):
      mu_c    = Wmu_c @ latent + bmu_c + x[c]
      sigma_c = softplus(Wsg_c @ latent + bsg_c)
      pi_c    = softmax(Wpi_c @ latent + bpi_c)   # over the 16 components

Strategy: shard H across the 8 cores (each core gets [4, 5, 64, 512]).
On-core, pixels are processed in supertiles of 4 groups x 2048 pixels;
each group's 32 latent channels occupy 32 SBUF partitions, so all
matmuls are dense 128-partition block-diagonal fp32r matmuls.
Head outputs are packed into chunks chosen so that (a) every chunk row
needs the same elementwise treatment, and (b) paired-head chunks are
g-major, which makes the store a single 128-partition [128, 2048] DMA
(64-partition DMAs run at half port width):
  A = [mu_r | mu_g] (g-major)   B = [sg_r | sg_g] (g-major)
  P = [pi_r | pi_g] (g-major)   M = [pi_b ; sg_b] (h-major halves)
  Mb = [mu_b] (64 rows)
mu residual+bias are folded into the matmuls via an augmented-x input
(5 channels + ones row).  softplus = Ln(Exp(z + b) + 1); the softmax
normalizer 1/s = Exp(-Ln(s)); all ACT functions live in one table set
(natural_log_exp_and_others) so the table is loaded exactly once.
"""

import sys

if "/opt/trn_rl_repo" not in sys.path:
    sys.path.insert(0, "/opt/trn_rl_repo")

import numpy as np

import concourse.bass as bass
import concourse.mybir as mybir
import concourse.tile as tile
from concourse import bacc
from concourse.bass_utils import run_bass_kernel_spmd

F32 = mybir.dt.float32
F32R = mybir.dt.float32r
AF = mybir.ActivationFunctionType
ALU = mybir.AluOpType

B, CIN, H, W = 4, 5, 512, 512
K, LAT = 16, 32
NCORES = 8
HC = H // NCORES            # 64 rows of H per core
PXB = HC * W                # 32768 pixels per batch image per core
G = 4                       # pixel groups per supertile
COLS = 2048                 # pixels per group per supertile
ST_PER_B = PXB // (G * COLS)  # supertiles per batch image (4)

_CACHE = {}


def _build_program(repeat=1, variant="full"):
    # variant: "full" | "nodma" (no output DMAs) | "dmaonly" (no compute)
    nc = bacc.Bacc("TRN2", target_bir_lowering=False, debug=False)

    xin = nc.dram_tensor("xin", [B, CIN + 1, PXB], F32R, kind="ExternalInput")

    wnames_r = {
        "lw1": [24, 128], "lw2": [128, 128], "lw3": [128, 128],
        "lA": [128, 128], "lB": [128, 128], "lP": [128, 128],
        "lM": [128, 128], "lE": [128, 64],
        "lrA": [24, 128], "lrE": [24, 64],
        "lsP": [128, 12], "lsM": [64, 12],
        "lbP": [12, 128], "lbM": [12, 64],
    }
    wnames_f = {
        "bb2": [128, 1], "bb3": [128, 1],
        "bB": [128, 1], "bP": [128, 1], "bM": [128, 1],
    }
    dram_w = {}
    for n, shp in wnames_r.items():
        dram_w[n] = nc.dram_tensor(n, shp, F32R, kind="ExternalInput")
    for n, shp in wnames_f.items():
        dram_w[n] = nc.dram_tensor(n, shp, F32, kind="ExternalInput")

    # paired outputs: channel = h*16 + k for the two heads in the chunk
    oA = nc.dram_tensor("oA", [B, 2 * K, PXB], F32, kind="ExternalOutput")
    oB = nc.dram_tensor("oB", [B, 2 * K, PXB], F32, kind="ExternalOutput")
    oP = nc.dram_tensor("oP", [B, 2 * K, PXB], F32R, kind="ExternalOutput")
    oM = nc.dram_tensor("oM", [B, 2 * K, PXB], F32R, kind="ExternalOutput")
    oMb = nc.dram_tensor("oMb", [B, K, PXB], F32, kind="ExternalOutput")

    from contextlib import ExitStack
    with tile.TileContext(nc) as tc, ExitStack() as es:
        consts = es.enter_context(tc.tile_pool(name="consts", bufs=1))
        xpool = es.enter_context(tc.tile_pool(name="xp", bufs=2))
        hpool = es.enter_context(tc.tile_pool(name="hp", bufs=2))
        spool = es.enter_context(tc.tile_pool(name="sp", bufs=2))
        tpool = es.enter_context(tc.tile_pool(name="tp", bufs=2))
        ps_z = es.enter_context(tc.tile_pool(name="psz", bufs=2, space="PSUM"))
        ps_sm = es.enter_context(tc.tile_pool(name="pssm", bufs=2,
                                              space="PSUM"))

        # --- load constants once ---
        wt = {}
        for n, shp in {**wnames_r, **wnames_f}.items():
            dt = F32R if n in wnames_r else F32
            t = consts.tile(shp, dt, tag=n)
            nc.sync.dma_start(out=t, in_=dram_w[n][:, :])
            wt[n] = t

        # DRAM views: [st] -> [g, hk, n] (g-major paired chunks)
        def view2(o, b_):
            return o[b_, :, :].rearrange("hk (s g n) -> s g hk n",
                                         s=ST_PER_B, g=G, n=COLS)

        def view1(o, b_, ch0=0):
            return o[b_, ch0:ch0 + K, :].rearrange("k (s g n) -> s g k n",
                                                   s=ST_PER_B, g=G, n=COLS)

        for rep_b in range(repeat * B):
            b_ = rep_b % B
            vA, vB, vP = view2(oA, b_), view2(oB, b_), view2(oP, b_)
            vMpi, vMsg = view1(oM, b_, 0), view1(oM, b_, K)
            vMb = view1(oMb, b_)
            for st in range(ST_PER_B):
                base = st * G * COLS
                x_t = xpool.tile([24, COLS], F32R, tag="x")
                nc.sync.dma_start(
                    out=x_t,
                    in_=xin[b_, :, base:base + G * COLS].rearrange(
                        "c (g n) -> g c n", n=COLS),
                )

                HW2 = COLS // 1024
                do_compute = variant != "dmaonly"
                do_outdma = variant != "nodma"

                sA = spool.tile([128, COLS], F32, tag="sA")
                sB = spool.tile([128, COLS], F32, tag="sB")
                sP = spool.tile([128, COLS], F32R, tag="sP")
                sM = spool.tile([128, COLS], F32R, tag="sM")
                sMb = spool.tile([64, COLS], F32, tag="sMb")

                def outs_dma():
                    nc.sync.dma_start(out=vA[st], in_=sA)
                    nc.sync.dma_start(out=vB[st], in_=sB)
                    nc.sync.dma_start(out=vP[st], in_=sP)
                    nc.sync.dma_start(out=vMpi[st], in_=sM[0:64, :])
                    nc.sync.dma_start(out=vMsg[st], in_=sM[64:128, :])
                    nc.sync.dma_start(out=vMb[st], in_=sMb)

                if not do_compute:
                    for _s in (sA, sB, sP, sM, sMb):
                        nc.vector.memset(_s[:, 0:1].bitcast(F32), 0.0)
                    outs_dma()
                    continue

                def zmm(lname, src, rows=128, res=None):
                    """[rows, 1024] psums filled by 512-col matmuls
                    (+ optional accumulated residual matmul from x_t)."""
                    outs = []
                    for hh in range(HW2):
                        z = ps_z.tile([rows, 1024], F32, tag="z")
                        for q2 in range(2):
                            cs = slice(hh * 1024 + q2 * 512,
                                       hh * 1024 + q2 * 512 + 512)
                            zs = slice(q2 * 512, q2 * 512 + 512)
                            nc.tensor.matmul(z[:, zs], wt[lname], src[:, cs],
                                             start=True, stop=res is None)
                            if res is not None:
                                nc.tensor.matmul(z[:, zs], wt[res],
                                                 x_t[:, cs],
                                                 start=False, stop=True)
                        outs.append(z)
                    return outs

                def hhs(hh):
                    return slice(hh * 1024, (hh + 1) * 1024)

                # --- backbone ---
                h1 = hpool.tile([128, COLS], F32R, tag="h1")
                for hh, z in enumerate(zmm("lw1", x_t)):
                    nc.vector.tensor_scalar(h1[:, hhs(hh)], z, 0.0, None,
                                            ALU.max)
                h2 = hpool.tile([128, COLS], F32R, tag="h2")
                for hh, z in enumerate(zmm("lw2", h1)):
                    nc.vector.tensor_scalar(h2[:, hhs(hh)], z, wt["bb2"], 0.0,
                                            ALU.add, ALU.max)
                lat = hpool.tile([128, COLS], F32R, tag="lat")
                for hh, z in enumerate(zmm("lw3", h2)):
                    nc.vector.tensor_scalar(lat[:, hhs(hh)], z, wt["bb3"], 0.0,
                                            ALU.add, ALU.max)

                # --- mu chunks (A: mu_r|mu_g g-major, Mb: mu_b) ---
                for hh, z in enumerate(zmm("lA", lat, res="lrA")):
                    nc.scalar.copy(sA[:, hhs(hh)], z)
                for hh, z in enumerate(zmm("lE", lat, rows=64, res="lrE")):
                    nc.vector.tensor_copy(sMb[:, hhs(hh)], z)

                # --- sg/pi z + exp ---
                for name, stile, btile in (("lB", sB, "bB"), ("lP", sP, "bP"),
                                           ("lM", sM, "bM")):
                    for hh, z in enumerate(zmm(name, lat)):
                        nc.scalar.activation(stile[:, hhs(hh)], z, AF.Exp,
                                             bias=wt[btile])

                # softplus finalize: ln(e + 1)
                nc.scalar.activation(sB, sB, AF.Ln, bias=1.0)
                nc.scalar.activation(sM[64:128, :], sM[64:128, :], AF.Ln,
                                     bias=1.0)

                # --- softmax normalize pi (P: pi_r|pi_g, M[0:64]: pi_b) ---
                for hh in range(HW2):
                    ss = ps_sm.tile([12, 1024], F32, tag="sm")
                    for q2 in range(2):
                        cs = slice(hh * 1024 + q2 * 512,
                                   hh * 1024 + q2 * 512 + 512)
                        zs = slice(q2 * 512, q2 * 512 + 512)
                        nc.tensor.matmul(ss[:, zs], wt["lsP"], sP[:, cs],
                                         start=True, stop=False)
                        nc.tensor.matmul(ss[:, zs], wt["lsM"], sM[0:64, cs],
                                         start=False, stop=True)
                    tl = tpool.tile([12, 1024], F32, tag="tl")
                    nc.scalar.activation(tl, ss, AF.Ln)
                    rs = tpool.tile([12, 1024], F32R, tag="rs")
                    nc.scalar.activation(rs, tl, AF.Exp, scale=-1.0)
                    bcP = ps_z.tile([128, 1024], F32, tag="z")
                    bcM = ps_z.tile([64, 1024], F32, tag="z")
                    for q2 in range(2):
                        zs = slice(q2 * 512, q2 * 512 + 512)
                        nc.tensor.matmul(bcP[:, zs], wt["lbP"], rs[:, zs],
                                         start=True, stop=True)
                        nc.tensor.matmul(bcM[:, zs], wt["lbM"], rs[:, zs],
                                         start=True, stop=True)
                    nc.vector.tensor_tensor(sP[:, hhs(hh)], sP[:, hhs(hh)],
                                            bcP, ALU.mult)
                    nc.vector.tensor_tensor(sM[0:64, hhs(hh)],
                                            sM[0:64, hhs(hh)], bcM, ALU.mult)

                if do_outdma:
                    outs_dma()

    # All ACT functions used (Exp, Ln, Copy) live in one table set; restrict
    # the chooser to it so the kernel performs a single ACT_TABLE_LOAD
    # instead of thrashing between exp/ln sets (~2.7us per reload).
    import concourse.bacc as bacc_mod
    orig_tables = bacc_mod.get_activation_tables
    def _only_nle(arch):
        t = orig_tables(arch)
        name = "natural_log_exp_and_others"
        if name not in t:
            return t
        return {k: (v if k == name else set()) for k, v in t.items()}
    bacc_mod.get_activation_tables = _only_nle
    try:
        nc.compile()
    finally:
        bacc_mod.get_activation_tables = orig_tables
    return nc


def _prep_weights(i):
    f = np.float32
    lw1 = np.zeros((24, 128), f)
    for g in range(G):
        lw1[6 * g:6 * g + 5, 32 * g:32 * (g + 1)] = i["w1"].T
        lw1[6 * g + 5, 32 * g:32 * (g + 1)] = i["b1"]
    lw2 = np.zeros((128, 128), f)
    lw3 = np.zeros((128, 128), f)
    for g in range(G):
        lw2[32 * g:32 * (g + 1), 32 * g:32 * (g + 1)] = i["w2"].T
        lw3[32 * g:32 * (g + 1), 32 * g:32 * (g + 1)] = i["w3"].T

    def pair_chunk(w0, w1):
        # g-major pair: out col = g*32 + h*16 + k
        l = np.zeros((128, 128), f)
        for g in range(G):
            l[32 * g:32 * (g + 1), 32 * g:32 * g + 16] = w0.T
            l[32 * g:32 * (g + 1), 32 * g + 16:32 * g + 32] = w1.T
        return l

    def half_chunks(w0, w1=None):
        # h-major: cols 0:64 head0 (g-major k), cols 64:128 head1
        ncol = 64 if w1 is None else 128
        l = np.zeros((128, ncol), f)
        for g in range(G):
            l[32 * g:32 * (g + 1), 16 * g:16 * (g + 1)] = w0.T
            if w1 is not None:
                l[32 * g:32 * (g + 1), 64 + 16 * g:64 + 16 * (g + 1)] = w1.T
        return l

    lA = pair_chunk(i["rmu_w"], i["gmu_w"])
    lB = pair_chunk(i["rsg_w"], i["gsg_w"])
    lP = pair_chunk(i["rpi_w"], i["gpi_w"])
    lM = half_chunks(i["bpi_w"], i["bsg_w"])
    lE = half_chunks(i["bmu_w"])

    lrA = np.zeros((24, 128), f)
    lrE = np.zeros((24, 64), f)
    for g in range(G):
        for k in range(K):
            lrA[6 * g + 0, 32 * g + k] = 1.0           # + x_r for mu_r
            lrA[6 * g + 5, 32 * g + k] = i["rmu_b"][k]
            lrA[6 * g + 1, 32 * g + 16 + k] = 1.0      # + x_g for mu_g
            lrA[6 * g + 5, 32 * g + 16 + k] = i["gmu_b"][k]
            lrE[6 * g + 2, 16 * g + k] = 1.0           # + x_b for mu_b
            lrE[6 * g + 5, 16 * g + k] = i["bmu_b"][k]

    # softmax sums: rows of ss = h*4 + g with h in (pi_r, pi_g, pi_b)
    lsP = np.zeros((128, 12), f)
    lsM = np.zeros((64, 12), f)
    lbP = np.zeros((12, 128), f)
    lbM = np.zeros((12, 64), f)
    for g in range(G):
        lsP[32 * g:32 * g + 16, 0 + g] = 1.0       # pi_r
        lsP[32 * g + 16:32 * g + 32, 4 + g] = 1.0  # pi_g
        lsM[16 * g:16 * (g + 1), 8 + g] = 1.0      # pi_b
        lbP[0 + g, 32 * g:32 * g + 16] = 1.0
        lbP[4 + g, 32 * g + 16:32 * g + 32] = 1.0
        lbM[8 + g, 16 * g:16 * (g + 1)] = 1.0

    col = lambda v: np.ascontiguousarray(v.reshape(-1, 1).astype(f))

    def pair_bias(b0, b1):
        v = np.zeros(128, f)
        for g in range(G):
            v[32 * g:32 * g + 16] = b0
            v[32 * g + 16:32 * g + 32] = b1
        return col(v)

    bb2 = col(np.tile(i["b2"], G))
    bb3 = col(np.tile(i["b3"], G))
    bB = pair_bias(i["rsg_b"], i["gsg_b"])
    bP = pair_bias(i["rpi_b"], i["gpi_b"])
    bM = col(np.concatenate([np.tile(i["bpi_b"], G), np.tile(i["bsg_b"], G)]))

    return {"lw1": lw1, "lw2": lw2, "lw3": lw3, "lA": lA, "lB": lB, "lP": lP,
            "lM": lM, "lE": lE, "lrA": lrA, "lrE": lrE, "lsP": lsP,
            "lsM": lsM, "lbP": lbP, "lbM": lbM, "bb2": bb2, "bb3": bb3,
            "bB": bB, "bP": bP, "bM": bM}


def _get_runner():
    """Compile the Bass program once and wrap it in a cached sharded jit."""
    if "runner" in _CACHE:
        return _CACHE["runner"]
    import jax
    from jax.sharding import Mesh, PartitionSpec
    from jax.experimental.shard_map import shard_map
    import concourse.mybir as mb
    import concourse.bass2jax as b2j

    nc = _CACHE.get("nc")
    if nc is None:
        nc = _CACHE["nc"] = _build_program()

    b2j.install_neuronx_cc_hook()
    partition_name = (nc.partition_id_tensor.name
                      if nc.partition_id_tensor else None)
    in_names, out_names, out_avals = [], [], []
    for alloc in nc.m.functions[0].allocations:
        if not isinstance(alloc, mb.MemoryLocationSet):
            continue
        name = alloc.memorylocations[0].name
        if alloc.kind == "ExternalInput":
            if name != partition_name:
                in_names.append(name)
        elif alloc.kind == "ExternalOutput":
            out_names.append(name)
            out_avals.append(jax.core.ShapedArray(
                tuple(alloc.tensor_shape), mb.dt.np(alloc.dtype)))
    n_params = len(in_names)
    bind_names = list(in_names + out_names)
    if partition_name is not None:
        bind_names.append(partition_name)
    bind_names = tuple(bind_names)

    def _body(*args):
        operands = list(args)
        if partition_name is not None:
            operands.append(b2j.partition_id_tensor())
        outs = b2j._bass_exec_p.bind(
            *operands,
            out_avals=tuple(out_avals),
            in_names=bind_names,
            out_names=tuple(out_names),
            lowering_input_output_aliases=(),
            sim_require_finite=True,
            sim_require_nnan=True,
            nc=nc,
        )
        return tuple(outs)

    devices = jax.devices()[:NCORES]
    mesh = Mesh(np.asarray(devices), ("core",))
    nin = n_params + len(out_names)
    fn = jax.jit(
        shard_map(_body, mesh=mesh,
                  in_specs=(PartitionSpec("core"),) * nin,
                  out_specs=(PartitionSpec("core"),) * len(out_names),
                  check_rep=False),
        keep_unused=True,
    )
    zeros = [np.zeros((NCORES * a.shape[0], *a.shape[1:]), a.dtype)
             for a in out_avals]
    runner = {"fn": fn, "in_names": in_names, "out_names": out_names,
              "out_avals": out_avals, "zeros": zeros, "mesh": mesh}
    _CACHE["runner"] = runner
    return runner


def _make_concat_inputs(inputs):
    wmaps = _prep_weights(inputs)
    x = inputs["x"]  # [B, 5, H, W]
    xs = []
    for c in range(NCORES):
        xc = x[:, :, c * HC:(c + 1) * HC, :].reshape(B, CIN, PXB)
        xa = np.empty((B, CIN + 1, PXB), np.float32)
        xa[:, :CIN] = xc
        xa[:, CIN] = 1.0
        xs.append(xa)
    per_core = {"xin": np.concatenate(xs, axis=0)}
    for n, w in wmaps.items():
        per_core[n] = np.concatenate([w] * NCORES, axis=0)
    return per_core


def kernel(**inputs):
    inputs = {k: np.asarray(v, dtype=np.float32) for k, v in inputs.items()}
    runner = _get_runner()
    concat = _make_concat_inputs(inputs)
    args = [concat[n] for n in runner["in_names"]]
    outs = runner["fn"](*args, *runner["zeros"])
    res = {}
    for name, aval, arr in zip(runner["out_names"], runner["out_avals"], outs):
        res[name] = np.asarray(arr).reshape(NCORES, *aval.shape)

    def gather(name, ch0):
        parts = [res[name][c][:, ch0:ch0 + K, :].reshape(B, K, HC, W)
                 for c in range(NCORES)]
        return np.concatenate(parts, axis=2)

    mu_r, mu_g = gather("oA", 0), gather("oA", K)
    sg_r, sg_g = gather("oB", 0), gather("oB", K)
    pi_r, pi_g = gather("oP", 0), gather("oP", K)
    pi_b, sg_b = gather("oM", 0), gather("oM", K)
    mu_b = gather("oMb", 0)
    return (mu_r, sg_r, pi_r, mu_g, sg_g, pi_g, mu_b, sg_b, pi_b)



# revision 8
# speedup vs baseline: 146.5109x; 146.5109x over previous
"""Trainium2 Bass kernel for per-pixel MDN head (nn_MDN_38946763440904).

Reference computation (per pixel, channels-first):
  h      = relu(W1 @ x5 + b1)        # 5  -> 32
  h      = relu(W2 @ h + b2)         # 32 -> 32
  latent = relu(W3 @ h + b3)         # 32 -> 32
  for c in (r, g, b):
      mu_c    = Wmu_c @ latent + bmu_c + x[c]
      sigma_c = softplus(Wsg_c @ latent + bsg_c)
      pi_c    = softmax(Wpi_c @ latent + bpi_c)   # over the 16 components

Strategy: shard H across the 8 cores (each core gets [4, 5, 64, 512]).
On-core, pixels are processed in supertiles of 4 groups x 2048 pixels;
each group's 32 latent channels occupy 32 SBUF partitions, so all
matmuls are dense 128-partition block-diagonal matmuls.  All matmul
operands and all DRAM outputs are bf16 (PSUM accumulation stays fp32);
this halves the HBM write traffic (the memory roofline for this
problem) and enables the PE fast-weight-load path.

Head outputs are packed into chunks so that every chunk needs one
uniform elementwise treatment:
  A  = [mu_r | mu_g] (g-major, 128 rows)     -> copy
  Bc = [sg_r | sg_g] (g-major)               -> exp, ln(1+e)
  P  = [pi_r | pi_g] (g-major)               -> exp, softmax-normalize
  M  = [pi_b ; sg_b] (64+64 rows)            -> exp, then ln on sg half
  Mb = [mu_b] column-folded to [128, 1024]   -> copy
mu residual+bias ride the head matmuls via an augmented-x input
(5 channels + ones row).  The softmax normalizer uses a fused
sum+broadcast matmul (block-diagonal ones weights produce the 16-wide
block sums already replicated to all 16 rows), a fast DVE reciprocal
(bit-trick + Newton, ~51 ULP), and one multiply.  Exp/Ln live in one
ACT table set (natural_log_exp_and_others) so the table loads once.
"""

import os
import sys

if "/opt/trn_rl_repo" not in sys.path:
    sys.path.insert(0, "/opt/trn_rl_repo")

import numpy as np
import ml_dtypes

import concourse.bass as bass
import concourse.mybir as mybir
import concourse.tile as tile
from concourse import bacc
from concourse.bass_utils import run_bass_kernel_spmd

F32 = mybir.dt.float32
BF16 = mybir.dt.bfloat16
AF = mybir.ActivationFunctionType
ALU = mybir.AluOpType

B, CIN, H, W = 4, 5, 512, 512
K, LAT = 16, 32
NCORES = 8
HC = H // NCORES            # 64 rows of H per core
PXB = HC * W                # 32768 pixels per batch image per core
G = 4                       # pixel groups per supertile
COLS = 2048                 # pixels per group per supertile
ST_PER_B = PXB // (G * COLS)  # supertiles per batch image (4)

NPBF = ml_dtypes.bfloat16

_CACHE = {}

# engine-assignment knobs (tuned from traces)
MULT_ENGINE = os.environ.get("MDN_MULT", "gps")     # gps | dve
COPY_ENGINE = os.environ.get("MDN_COPY", "act")     # act | dve


def _build_program(variant="full"):
    nc = bacc.Bacc("TRN2", target_bir_lowering=False, debug=False)

    xin = nc.dram_tensor("xin", [B, CIN + 1, PXB], BF16, kind="ExternalInput")

    wnames_b = {
        "lw1": [24, 128], "lw2": [128, 128], "lw3": [128, 128],
        "lA": [128, 128], "lB": [128, 128], "lP": [128, 128],
        "lM": [128, 128], "lE": [128, 64],
        "lrA": [24, 128], "lrE": [24, 64],
        "lsPbig": [128, 128], "lsMbig": [64, 64],
    }
    wnames_f = {
        "bb2": [128, 1], "bb3": [128, 1],
        "bB": [128, 1], "bP": [128, 1], "bM": [128, 1],
    }
    dram_w = {}
    for n, shp in wnames_b.items():
        dram_w[n] = nc.dram_tensor(n, shp, BF16, kind="ExternalInput")
    for n, shp in wnames_f.items():
        dram_w[n] = nc.dram_tensor(n, shp, F32, kind="ExternalInput")

    # paired outputs: channel = h*16 + k for the two heads in the chunk
    oA = nc.dram_tensor("oA", [B, 2 * K, PXB], BF16, kind="ExternalOutput")
    oB = nc.dram_tensor("oB", [B, 2 * K, PXB], BF16, kind="ExternalOutput")
    oP = nc.dram_tensor("oP", [B, 2 * K, PXB], BF16, kind="ExternalOutput")
    oM = nc.dram_tensor("oM", [B, 2 * K, PXB], BF16, kind="ExternalOutput")
    oMb = nc.dram_tensor("oMb", [B, K, PXB], BF16, kind="ExternalOutput")

    from contextlib import ExitStack
    with tile.TileContext(nc) as tc, ExitStack() as es:
        consts = es.enter_context(tc.tile_pool(name="consts", bufs=1))
        xpool = es.enter_context(tc.tile_pool(name="xp", bufs=2))
        hpool = es.enter_context(tc.tile_pool(name="hp", bufs=2))
        spool = es.enter_context(tc.tile_pool(name="sp", bufs=2))
        rpool = es.enter_context(tc.tile_pool(name="rp", bufs=2))
        ps_z = es.enter_context(tc.tile_pool(name="psz", bufs=2, space="PSUM"))
        ps_bc = es.enter_context(tc.tile_pool(name="psbc", bufs=2,
                                              space="PSUM"))

        # --- load constants once ---
        wt = {}
        for n, shp in {**wnames_b, **wnames_f}.items():
            dt = BF16 if n in wnames_b else F32
            t = consts.tile(shp, dt, tag=n, name=n)
            nc.sync.dma_start(out=t, in_=dram_w[n][:, :])
            wt[n] = t

        # DRAM views: [st] -> [g, hk, n] (g-major paired chunks)
        def view2(o, b_):
            return o[b_, :, :].rearrange("hk (s g n) -> s g hk n",
                                         s=ST_PER_B, g=G, n=COLS)

        def view1(o, b_, ch0=0):
            return o[b_, ch0:ch0 + K, :].rearrange("k (s g n) -> s g k n",
                                                   s=ST_PER_B, g=G, n=COLS)

        for b_ in range(B):
            vA, vB, vP = view2(oA, b_), view2(oB, b_), view2(oP, b_)
            vMpi, vMsg = view1(oM, b_, 0), view1(oM, b_, K)
            vMb = oMb[b_, :, :].rearrange("k (s g n) -> s g k n",
                                          s=ST_PER_B, g=G, n=COLS)
            for st in range(ST_PER_B):
                base = st * G * COLS
                x_t = xpool.tile([24, COLS], BF16, tag="x", name="x_t")
                nc.sync.dma_start(
                    out=x_t,
                    in_=xin[b_, :, base:base + G * COLS].rearrange(
                        "c (g n) -> g c n", n=COLS),
                )

                sA = spool.tile([128, COLS], BF16, tag="sA", name="sA")
                sB = spool.tile([128, COLS], BF16, tag="sB", name="sB")
                sP = spool.tile([128, COLS], BF16, tag="sP", name="sP")
                sM = spool.tile([128, COLS], BF16, tag="sM", name="sM")
                sMb = spool.tile([64, COLS], BF16, tag="sMb", name="sMb")

                def layer(dst, wname, src, bias=None):
                    """backbone layer: 2 half-tiles of [128,1024] psum,
                    relu(+bias) evacuated on DVE into dst (bf16)."""
                    for hh in range(2):
                        z = ps_z.tile([128, 1024], F32, tag="z", name="z")
                        for q in range(2):
                            cs = slice(hh * 1024 + q * 512,
                                       hh * 1024 + q * 512 + 512)
                            zs = slice(q * 512, q * 512 + 512)
                            nc.tensor.matmul(z[:, zs], wt[wname], src[:, cs],
                                             start=True, stop=True)
                        dsl = dst[:, hh * 1024:(hh + 1) * 1024]
                        if bias is None:
                            nc.vector.tensor_scalar(dsl, z, 0.0, None,
                                                    ALU.max)
                        else:
                            nc.vector.tensor_scalar(dsl, z, wt[bias], 0.0,
                                                    ALU.add, ALU.max)

                # --- backbone ---
                h1 = hpool.tile([128, COLS], BF16, tag="h1", name="h1")
                layer(h1, "lw1", x_t)
                h2 = hpool.tile([128, COLS], BF16, tag="h2", name="h2")
                layer(h2, "lw2", h1, bias="bb2")
                lat = hpool.tile([128, COLS], BF16, tag="lat", name="lat")
                layer(lat, "lw3", h2, bias="bb3")

                copy_eng = nc.scalar if COPY_ENGINE == "act" else nc.vector

                # --- mu chunk A = [mu_r | mu_g] (residual via lrA) ---
                for hh in range(2):
                    z = ps_z.tile([128, 1024], F32, tag="z", name="z")
                    for q in range(2):
                        cs = slice(hh * 1024 + q * 512,
                                   hh * 1024 + q * 512 + 512)
                        zs = slice(q * 512, q * 512 + 512)
                        nc.tensor.matmul(z[:, zs], wt["lA"], lat[:, cs],
                                         start=True, stop=False)
                    for q in range(2):
                        cs = slice(hh * 1024 + q * 512,
                                   hh * 1024 + q * 512 + 512)
                        zs = slice(q * 512, q * 512 + 512)
                        nc.tensor.matmul(z[:, zs], wt["lrA"], x_t[:, cs],
                                         start=False, stop=True)
                    if COPY_ENGINE == "act":
                        nc.scalar.copy(sA[:, hh * 1024:(hh + 1) * 1024], z)
                    else:
                        nc.vector.tensor_copy(
                            sA[:, hh * 1024:(hh + 1) * 1024], z)

                # --- mu_b chunk [64, 2048] (residual via lrE) ---
                for hh in range(2):
                    zmb = ps_z.tile([128, 1024], F32, tag="z", name="zmb")
                    for q in range(2):
                        cs = slice(hh * 1024 + q * 512,
                                   hh * 1024 + q * 512 + 512)
                        zs = slice(q * 512, q * 512 + 512)
                        nc.tensor.matmul(zmb[0:64, zs], wt["lE"],
                                         lat[:, cs], start=True, stop=False)
                    for q in range(2):
                        cs = slice(hh * 1024 + q * 512,
                                   hh * 1024 + q * 512 + 512)
                        zs = slice(q * 512, q * 512 + 512)
                        nc.tensor.matmul(zmb[0:64, zs], wt["lrE"],
                                         x_t[:, cs], start=False, stop=True)
                    dsl = sMb[:, hh * 1024:(hh + 1) * 1024]
                    if COPY_ENGINE == "act":
                        nc.scalar.copy(dsl, zmb[0:64, :])
                    else:
                        nc.vector.tensor_copy(dsl, zmb[0:64, :])

                # --- sg/pi chunks: z then exp (bias pre-exp) ---
                for name, stile, btile in (("lB", sB, "bB"), ("lP", sP, "bP"),
                                           ("lM", sM, "bM")):
                    for hh in range(2):
                        z = ps_z.tile([128, 1024], F32, tag="z", name="z")
                        for q in range(2):
                            cs = slice(hh * 1024 + q * 512,
                                       hh * 1024 + q * 512 + 512)
                            zs = slice(q * 512, q * 512 + 512)
                            nc.tensor.matmul(z[:, zs], wt[name], lat[:, cs],
                                             start=True, stop=True)
                        nc.scalar.activation(
                            stile[:, hh * 1024:(hh + 1) * 1024], z, AF.Exp,
                            bias=wt[btile])

                # softplus finalize: ln(e + 1)
                nc.scalar.activation(sB, sB, AF.Ln, bias=1.0)
                nc.scalar.activation(sM[64:128, :], sM[64:128, :], AF.Ln,
                                     bias=1.0)

                # --- softmax normalize: fused sum+broadcast matmul,
                #     fast reciprocal, multiply ---
                rbcP = rpool.tile([128, COLS], F32, tag="rbcP", name="rbcP")
                rbcM = rpool.tile([64, COLS], F32, tag="rbcM", name="rbcM")
                for hh in range(2):
                    bcp = ps_bc.tile([128, 1024], F32, tag="bc", name="bcp")
                    for q in range(2):
                        cs = slice(hh * 1024 + q * 512,
                                   hh * 1024 + q * 512 + 512)
                        zs = slice(q * 512, q * 512 + 512)
                        nc.tensor.matmul(bcp[:, zs], wt["lsPbig"], sP[:, cs],
                                         start=True, stop=True)
                    nc.vector.reciprocal_approx_fast(
                        rbcP[:, hh * 1024:(hh + 1) * 1024], bcp)
                    bcm = ps_bc.tile([128, 1024], F32, tag="bc", name="bcm")
                    for q in range(2):
                        cs = slice(hh * 1024 + q * 512,
                                   hh * 1024 + q * 512 + 512)
                        zs = slice(q * 512, q * 512 + 512)
                        nc.tensor.matmul(bcm[0:64, zs], wt["lsMbig"],
                                         sM[0:64, cs], start=True, stop=True)
                    nc.vector.reciprocal_approx_fast(
                        rbcM[:, hh * 1024:(hh + 1) * 1024], bcm[0:64, :])

                mult_eng = nc.gpsimd if MULT_ENGINE == "gps" else nc.vector
                mult_eng.tensor_tensor(sP, sP, rbcP, ALU.mult)
                mult_eng.tensor_tensor(sM[0:64, :], sM[0:64, :], rbcM,
                                       ALU.mult)

                nc.sync.dma_start(out=vA[st], in_=sA)
                nc.sync.dma_start(out=vB[st], in_=sB)
                nc.sync.dma_start(out=vP[st], in_=sP)
                nc.sync.dma_start(out=vMpi[st], in_=sM[0:64, :])
                nc.sync.dma_start(out=vMsg[st], in_=sM[64:128, :])
                nc.sync.dma_start(out=vMb[st], in_=sMb)

    # All ACT functions used (Exp, Ln, Copy) live in one table set; restrict
    # the chooser to it so the kernel performs a single ACT_TABLE_LOAD
    # instead of thrashing between exp/ln sets (~2.7us per reload).
    import concourse.bacc as bacc_mod
    orig_tables = bacc_mod.get_activation_tables
    def _only_nle(arch):
        t = orig_tables(arch)
        name = "natural_log_exp_and_others"
        if name not in t:
            return t
        return {k: (v if k == name else set()) for k, v in t.items()}
    bacc_mod.get_activation_tables = _only_nle
    try:
        nc.compile()
    finally:
        bacc_mod.get_activation_tables = orig_tables
    return nc


def _prep_weights(i):
    f = np.float32
    lw1 = np.zeros((24, 128), f)
    for g in range(G):
        lw1[6 * g:6 * g + 5, 32 * g:32 * (g + 1)] = i["w1"].T
        lw1[6 * g + 5, 32 * g:32 * (g + 1)] = i["b1"]
    lw2 = np.zeros((128, 128), f)
    lw3 = np.zeros((128, 128), f)
    for g in range(G):
        lw2[32 * g:32 * (g + 1), 32 * g:32 * (g + 1)] = i["w2"].T
        lw3[32 * g:32 * (g + 1), 32 * g:32 * (g + 1)] = i["w3"].T

    def pair_chunk(w0, w1):
        # g-major pair: out col = g*32 + h*16 + k
        l = np.zeros((128, 128), f)
        for g in range(G):
            l[32 * g:32 * (g + 1), 32 * g:32 * g + 16] = w0.T
            l[32 * g:32 * (g + 1), 32 * g + 16:32 * g + 32] = w1.T
        return l

    def half_chunks(w0, w1=None):
        # h-major: cols 0:64 head0 (g-major k), cols 64:128 head1
        ncol = 64 if w1 is None else 128
        l = np.zeros((128, ncol), f)
        for g in range(G):
            l[32 * g:32 * (g + 1), 16 * g:16 * (g + 1)] = w0.T
            if w1 is not None:
                l[32 * g:32 * (g + 1), 64 + 16 * g:64 + 16 * (g + 1)] = w1.T
        return l

    lA = pair_chunk(i["rmu_w"], i["gmu_w"])
    lB = pair_chunk(i["rsg_w"], i["gsg_w"])
    lP = pair_chunk(i["rpi_w"], i["gpi_w"])
    lM = half_chunks(i["bpi_w"], i["bsg_w"])
    lE = half_chunks(i["bmu_w"])

    lrA = np.zeros((24, 128), f)
    lrE = np.zeros((24, 64), f)
    for g in range(G):
        for k in range(K):
            lrA[6 * g + 0, 32 * g + k] = 1.0           # + x_r for mu_r
            lrA[6 * g + 5, 32 * g + k] = i["rmu_b"][k]
            lrA[6 * g + 1, 32 * g + 16 + k] = 1.0      # + x_g for mu_g
            lrA[6 * g + 5, 32 * g + 16 + k] = i["gmu_b"][k]
            lrE[6 * g + 2, 16 * g + k] = 1.0           # + x_b for mu_b
            lrE[6 * g + 5, 16 * g + k] = i["bmu_b"][k]

    # fused softmax sum+broadcast: 16x16 ones blocks on the diagonal
    blk = np.ones((16, 16), f)
    lsPbig = np.kron(np.eye(8, dtype=f), blk)          # [128, 128]
    lsMbig = np.kron(np.eye(4, dtype=f), blk)          # [64, 64]

    col = lambda v: np.ascontiguousarray(v.reshape(-1, 1).astype(f))

    def pair_bias(b0, b1):
        v = np.zeros(128, f)
        for g in range(G):
            v[32 * g:32 * g + 16] = b0
            v[32 * g + 16:32 * g + 32] = b1
        return col(v)

    bb2 = col(np.tile(i["b2"], G))
    bb3 = col(np.tile(i["b3"], G))
    bB = pair_bias(i["rsg_b"], i["gsg_b"])
    bP = pair_bias(i["rpi_b"], i["gpi_b"])
    bM = col(np.concatenate([np.tile(i["bpi_b"], G), np.tile(i["bsg_b"], G)]))

    w = {"lw1": lw1, "lw2": lw2, "lw3": lw3, "lA": lA, "lB": lB, "lP": lP,
         "lM": lM, "lE": lE, "lrA": lrA, "lrE": lrE,
         "lsPbig": lsPbig, "lsMbig": lsMbig}
    w = {k: v.astype(NPBF) for k, v in w.items()}
    w.update({"bb2": bb2, "bb3": bb3, "bB": bB, "bP": bP, "bM": bM})
    return w


def _get_runner():
    """Compile the Bass program once and wrap it in a cached sharded jit."""
    if "runner" in _CACHE:
        return _CACHE["runner"]
    import jax
    from jax.sharding import Mesh, PartitionSpec
    from jax.experimental.shard_map import shard_map
    import concourse.mybir as mb
    import concourse.bass2jax as b2j

    nc = _CACHE.get("nc")
    if nc is None:
        nc = _CACHE["nc"] = _build_program()

    b2j.install_neuronx_cc_hook()
    partition_name = (nc.partition_id_tensor.name
                      if nc.partition_id_tensor else None)
    in_names, out_names, out_avals = [], [], []
    for alloc in nc.m.functions[0].allocations:
        if not isinstance(alloc, mb.MemoryLocationSet):
            continue
        name = alloc.memorylocations[0].name
        if alloc.kind == "ExternalInput":
            if name != partition_name:
                in_names.append(name)
        elif alloc.kind == "ExternalOutput":
            out_names.append(name)
            out_avals.append(jax.core.ShapedArray(
                tuple(alloc.tensor_shape), mb.dt.np(alloc.dtype)))
    n_params = len(in_names)
    bind_names = list(in_names + out_names)
    if partition_name is not None:
        bind_names.append(partition_name)
    bind_names = tuple(bind_names)

    def _body(*args):
        operands = list(args)
        if partition_name is not None:
            operands.append(b2j.partition_id_tensor())
        outs = b2j._bass_exec_p.bind(
            *operands,
            out_avals=tuple(out_avals),
            in_names=bind_names,
            out_names=tuple(out_names),
            lowering_input_output_aliases=(),
            sim_require_finite=True,
            sim_require_nnan=True,
            nc=nc,
        )
        return tuple(outs)

    devices = jax.devices()[:NCORES]
    mesh = Mesh(np.asarray(devices), ("core",))
    nin = n_params + len(out_names)
    fn = jax.jit(
        shard_map(_body, mesh=mesh,
                  in_specs=(PartitionSpec("core"),) * nin,
                  out_specs=(PartitionSpec("core"),) * len(out_names),
                  check_rep=False),
        keep_unused=True,
    )
    zeros = [np.zeros((NCORES * a.shape[0], *a.shape[1:]), a.dtype)
             for a in out_avals]
    runner = {"fn": fn, "in_names": in_names, "out_names": out_names,
              "out_avals": out_avals, "zeros": zeros, "mesh": mesh}
    _CACHE["runner"] = runner
    return runner


def _make_concat_inputs(inputs):
    wmaps = _prep_weights(inputs)
    x = inputs["x"]  # [B, 5, H, W]
    xs = []
    for c in range(NCORES):
        xc = x[:, :, c * HC:(c + 1) * HC, :].reshape(B, CIN, PXB)
        xa = np.empty((B, CIN + 1, PXB), np.float32)
        xa[:, :CIN] = xc
        xa[:, CIN] = 1.0
        xs.append(xa.astype(NPBF))
    per_core = {"xin": np.concatenate(xs, axis=0)}
    for n, w in wmaps.items():
        per_core[n] = np.concatenate([w] * NCORES, axis=0)
    return per_core


def kernel(**inputs):
    inputs = {k: np.asarray(v, dtype=np.float32) for k, v in inputs.items()}
    runner = _get_runner()
    concat = _make_concat_inputs(inputs)
    args = [concat[n] for n in runner["in_names"]]
    outs = runner["fn"](*args, *runner["zeros"])
    res = {}
    for name, aval, arr in zip(runner["out_names"], runner["out_avals"], outs):
        res[name] = np.asarray(arr).reshape(NCORES, *aval.shape)

    def gather(name, ch0):
        parts = [res[name][c][:, ch0:ch0 + K, :].astype(np.float32)
                 .reshape(B, K, HC, W) for c in range(NCORES)]
        return np.concatenate(parts, axis=2)

    mu_r, mu_g = gather("oA", 0), gather("oA", K)
    sg_r, sg_g = gather("oB", 0), gather("oB", K)
    pi_r, pi_g = gather("oP", 0), gather("oP", K)
    pi_b, sg_b = gather("oM", 0), gather("oM", K)

    # oMb is column-folded: partition = 64*f + 16*g + k over (s g f n)
    mb_parts = []
    for c in range(NCORES):
        a = res["oMb"][c].astype(np.float32)  # [B, K, PXB]
        mb_parts.append(a.reshape(B, K, HC, W))
    mu_b = np.concatenate(mb_parts, axis=2)
    return (mu_r, sg_r, pi_r, mu_g, sg_g, pi_g, mu_b, sg_b, pi_b)


# revision 11
# speedup vs baseline: 179.3161x; 1.2239x over previous
"""Trainium2 Bass kernel for per-pixel MDN head (nn_MDN_38946763440904).

Reference computation (per pixel, channels-first):
  h      = relu(W1 @ x5 + b1)        # 5  -> 32
  h      = relu(W2 @ h + b2)         # 32 -> 32
  latent = relu(W3 @ h + b3)         # 32 -> 32
  for c in (r, g, b):
      mu_c    = Wmu_c @ latent + bmu_c + x[c]
      sigma_c = softplus(Wsg_c @ latent + bsg_c)
      pi_c    = softmax(Wpi_c @ latent + bpi_c)   # over the 16 components

Strategy: shard H across the 8 cores (each core gets [4, 5, 64, 512]).
On-core, pixels are processed in supertiles of 4 groups x 2048 pixels;
each group's 32 latent channels occupy 32 SBUF partitions, so all
matmuls are dense 128-partition block-diagonal matmuls.  All matmul
operands and all DRAM outputs are bf16 (PSUM accumulation stays fp32);
this halves the HBM write traffic (the memory roofline for this
problem) and enables the PE fast-weight-load path.

Head outputs are packed into chunks so that every chunk needs one
uniform elementwise treatment:
  A  = [mu_r | mu_g] (g-major, 128 rows)     -> copy
  Bc = [sg_r | sg_g] (g-major)               -> exp, ln(1+e)
  P  = [pi_r | pi_g] (g-major)               -> exp, softmax-normalize
  M  = [pi_b ; sg_b] (64+64 rows)            -> exp, then ln on sg half
  Mb = [mu_b] column-folded to [128, 1024]   -> copy
mu residual+bias ride the head matmuls via an augmented-x input
(5 channels + ones row).  The softmax normalizer uses a fused
sum+broadcast matmul (block-diagonal ones weights produce the 16-wide
block sums already replicated to all 16 rows), a fast DVE reciprocal
(bit-trick + Newton, ~51 ULP), and one multiply.  Exp/Ln live in one
ACT table set (natural_log_exp_and_others) so the table loads once.
"""

import os
import sys

if "/opt/trn_rl_repo" not in sys.path:
    sys.path.insert(0, "/opt/trn_rl_repo")

import numpy as np
import ml_dtypes

import concourse.bass as bass
import concourse.mybir as mybir
import concourse.tile as tile
from concourse import bacc
from concourse.bass_utils import run_bass_kernel_spmd

F32 = mybir.dt.float32
BF16 = mybir.dt.bfloat16
AF = mybir.ActivationFunctionType
ALU = mybir.AluOpType

B, CIN, H, W = 4, 5, 512, 512
K, LAT = 16, 32
NCORES = 8
HC = H // NCORES            # 64 rows of H per core
PXB = HC * W                # 32768 pixels per batch image per core
G = 4                       # pixel groups per supertile
COLS = 2048                 # pixels per group per supertile
ST_PER_B = PXB // (G * COLS)  # supertiles per batch image (4)

NPBF = ml_dtypes.bfloat16

_CACHE = {}

# engine-assignment knobs (tuned from traces)
MULT_ENGINE = os.environ.get("MDN_MULT", "split")   # gps | dve | split
COPY_ENGINE = os.environ.get("MDN_COPY", "act")     # act | dve


def _build_program(variant="full"):
    nc = bacc.Bacc("TRN2", target_bir_lowering=False, debug=False)

    xin = nc.dram_tensor("xin", [B, CIN + 1, PXB], BF16, kind="ExternalInput")

    wnames_b = {
        "lw1": [24, 128], "lw2": [128, 128], "lw3": [128, 128],
        "lA": [128, 128], "lB": [128, 128], "lP": [128, 128],
        "lM": [128, 128], "lE": [128, 64],
        "lrA": [24, 128], "lrE": [24, 64],
        "lsPbig": [128, 128], "lsMbig": [64, 64],
    }
    wnames_f = {
        "bb2": [128, 1], "bb3": [128, 1],
        "bB": [128, 1], "bP": [128, 1], "bM": [128, 1],
    }
    dram_w = {}
    for n, shp in wnames_b.items():
        dram_w[n] = nc.dram_tensor(n, shp, BF16, kind="ExternalInput")
    for n, shp in wnames_f.items():
        dram_w[n] = nc.dram_tensor(n, shp, F32, kind="ExternalInput")

    # paired outputs: channel = h*16 + k for the two heads in the chunk
    oA = nc.dram_tensor("oA", [B, 2 * K, PXB], BF16, kind="ExternalOutput")
    oB = nc.dram_tensor("oB", [B, 2 * K, PXB], BF16, kind="ExternalOutput")
    oP = nc.dram_tensor("oP", [B, 2 * K, PXB], BF16, kind="ExternalOutput")
    oM = nc.dram_tensor("oM", [B, 2 * K, PXB], BF16, kind="ExternalOutput")
    oMb = nc.dram_tensor("oMb", [B, K, PXB], BF16, kind="ExternalOutput")

    from contextlib import ExitStack
    with tile.TileContext(nc) as tc, ExitStack() as es:
        consts = es.enter_context(tc.tile_pool(name="consts", bufs=1))
        xpool = es.enter_context(tc.tile_pool(name="xp", bufs=2))
        hpool = es.enter_context(tc.tile_pool(name="hp", bufs=2))
        spool = es.enter_context(tc.tile_pool(name="sp", bufs=2))
        rpool = es.enter_context(tc.tile_pool(name="rp", bufs=2))
        ps_z = es.enter_context(tc.tile_pool(name="psz", bufs=2, space="PSUM"))
        ps_bc = es.enter_context(tc.tile_pool(name="psbc", bufs=2,
                                              space="PSUM"))

        # --- load constants once ---
        wt = {}
        for n, shp in {**wnames_b, **wnames_f}.items():
            dt = BF16 if n in wnames_b else F32
            t = consts.tile(shp, dt, tag=n, name=n)
            nc.sync.dma_start(out=t, in_=dram_w[n][:, :])
            wt[n] = t

        # DRAM views: [st] -> [g, hk, n] (g-major paired chunks)
        def view2(o, b_):
            return o[b_, :, :].rearrange("hk (s g n) -> s g hk n",
                                         s=ST_PER_B, g=G, n=COLS)

        def view1(o, b_, ch0=0):
            return o[b_, ch0:ch0 + K, :].rearrange("k (s g n) -> s g k n",
                                                   s=ST_PER_B, g=G, n=COLS)

        for b_ in range(B):
            vA, vB, vP = view2(oA, b_), view2(oB, b_), view2(oP, b_)
            vMpi, vMsg = view1(oM, b_, 0), view1(oM, b_, K)
            vMb = oMb[b_, :, :].rearrange("k (s g n) -> s g k n",
                                          s=ST_PER_B, g=G, n=COLS)
            # prefetch all x tiles for this image on the ACT HWDGE queue so
            # input loads never queue behind the output stores on Sync
            x_ts = []
            for st in range(ST_PER_B):
                base = st * G * COLS
                x_t = xpool.tile([24, COLS], BF16, tag="x", name="x_t",
                                 bufs=ST_PER_B + 1)
                nc.scalar.dma_start(
                    out=x_t,
                    in_=xin[b_, :, base:base + G * COLS].rearrange(
                        "c (g n) -> g c n", n=COLS),
                )
                x_ts.append(x_t)
            for st in range(ST_PER_B):
                x_t = x_ts[st]

                sA = spool.tile([128, COLS], BF16, tag="sA", name="sA")
                sB = spool.tile([128, COLS], BF16, tag="sB", name="sB")
                sP = spool.tile([128, COLS], BF16, tag="sP", name="sP")
                sM = spool.tile([128, COLS], BF16, tag="sM", name="sM")
                sMb = spool.tile([64, COLS], BF16, tag="sMb", name="sMb")

                def layer(dst, wname, src, bias=None):
                    """backbone layer: 2 half-tiles of [128,1024] psum,
                    relu(+bias) evacuated on DVE into dst (bf16)."""
                    for hh in range(2):
                        z = ps_z.tile([128, 1024], F32, tag="z", name="z")
                        for q in range(2):
                            cs = slice(hh * 1024 + q * 512,
                                       hh * 1024 + q * 512 + 512)
                            zs = slice(q * 512, q * 512 + 512)
                            nc.tensor.matmul(z[:, zs], wt[wname], src[:, cs],
                                             start=True, stop=True)
                        dsl = dst[:, hh * 1024:(hh + 1) * 1024]
                        if bias is None:
                            nc.vector.tensor_scalar(dsl, z, 0.0, None,
                                                    ALU.max)
                        else:
                            nc.vector.tensor_scalar(dsl, z, wt[bias], 0.0,
                                                    ALU.add, ALU.max)

                # --- backbone ---
                h1 = hpool.tile([128, COLS], BF16, tag="h1", name="h1")
                layer(h1, "lw1", x_t)
                h2 = hpool.tile([128, COLS], BF16, tag="h2", name="h2")
                layer(h2, "lw2", h1, bias="bb2")
                lat = hpool.tile([128, COLS], BF16, tag="lat", name="lat")
                layer(lat, "lw3", h2, bias="bb3")

                copy_eng = nc.scalar if COPY_ENGINE == "act" else nc.vector

                # --- mu chunk A = [mu_r | mu_g] (residual via lrA) ---
                for hh in range(2):
                    z = ps_z.tile([128, 1024], F32, tag="z", name="z")
                    for q in range(2):
                        cs = slice(hh * 1024 + q * 512,
                                   hh * 1024 + q * 512 + 512)
                        zs = slice(q * 512, q * 512 + 512)
                        nc.tensor.matmul(z[:, zs], wt["lA"], lat[:, cs],
                                         start=True, stop=False)
                    for q in range(2):
                        cs = slice(hh * 1024 + q * 512,
                                   hh * 1024 + q * 512 + 512)
                        zs = slice(q * 512, q * 512 + 512)
                        nc.tensor.matmul(z[:, zs], wt["lrA"], x_t[:, cs],
                                         start=False, stop=True)
                    if COPY_ENGINE == "act":
                        nc.scalar.copy(sA[:, hh * 1024:(hh + 1) * 1024], z)
                    else:
                        nc.vector.tensor_copy(
                            sA[:, hh * 1024:(hh + 1) * 1024], z)

                # --- mu_b chunk [64, 2048] (residual via lrE) ---
                for hh in range(2):
                    zmb = ps_z.tile([128, 1024], F32, tag="z", name="zmb")
                    for q in range(2):
                        cs = slice(hh * 1024 + q * 512,
                                   hh * 1024 + q * 512 + 512)
                        zs = slice(q * 512, q * 512 + 512)
                        nc.tensor.matmul(zmb[0:64, zs], wt["lE"],
                                         lat[:, cs], start=True, stop=False)
                    for q in range(2):
                        cs = slice(hh * 1024 + q * 512,
                                   hh * 1024 + q * 512 + 512)
                        zs = slice(q * 512, q * 512 + 512)
                        nc.tensor.matmul(zmb[0:64, zs], wt["lrE"],
                                         x_t[:, cs], start=False, stop=True)
                    dsl = sMb[:, hh * 1024:(hh + 1) * 1024]
                    if COPY_ENGINE == "act":
                        nc.scalar.copy(dsl, zmb[0:64, :])
                    else:
                        nc.vector.tensor_copy(dsl, zmb[0:64, :])

                # --- sg/pi chunks: z then exp (bias pre-exp) ---
                for name, stile, btile in (("lB", sB, "bB"), ("lP", sP, "bP"),
                                           ("lM", sM, "bM")):
                    for hh in range(2):
                        z = ps_z.tile([128, 1024], F32, tag="z", name="z")
                        for q in range(2):
                            cs = slice(hh * 1024 + q * 512,
                                       hh * 1024 + q * 512 + 512)
                            zs = slice(q * 512, q * 512 + 512)
                            nc.tensor.matmul(z[:, zs], wt[name], lat[:, cs],
                                             start=True, stop=True)
                        nc.scalar.activation(
                            stile[:, hh * 1024:(hh + 1) * 1024], z, AF.Exp,
                            bias=wt[btile])

                # softplus finalize: ln(e + 1)
                nc.scalar.activation(sB, sB, AF.Ln, bias=1.0)
                nc.scalar.activation(sM[64:128, :], sM[64:128, :], AF.Ln,
                                     bias=1.0)

                # --- softmax normalize: fused sum+broadcast matmul,
                #     fast reciprocal, multiply ---
                rbcP = rpool.tile([128, COLS], F32, tag="rbcP", name="rbcP")
                rbcM = rpool.tile([64, COLS], F32, tag="rbcM", name="rbcM")
                for hh in range(2):
                    bcp = ps_bc.tile([128, 1024], F32, tag="bc", name="bcp")
                    for q in range(2):
                        cs = slice(hh * 1024 + q * 512,
                                   hh * 1024 + q * 512 + 512)
                        zs = slice(q * 512, q * 512 + 512)
                        nc.tensor.matmul(bcp[:, zs], wt["lsPbig"], sP[:, cs],
                                         start=True, stop=True)
                    nc.vector.reciprocal_approx_fast(
                        rbcP[:, hh * 1024:(hh + 1) * 1024], bcp)
                    bcm = ps_bc.tile([128, 1024], F32, tag="bc", name="bcm")
                    for q in range(2):
                        cs = slice(hh * 1024 + q * 512,
                                   hh * 1024 + q * 512 + 512)
                        zs = slice(q * 512, q * 512 + 512)
                        nc.tensor.matmul(bcm[0:64, zs], wt["lsMbig"],
                                         sM[0:64, cs], start=True, stop=True)
                    nc.vector.reciprocal_approx_fast(
                        rbcM[:, hh * 1024:(hh + 1) * 1024], bcm[0:64, :])

                if MULT_ENGINE == "gps":
                    nc.gpsimd.tensor_tensor(sP, sP, rbcP, ALU.mult)
                    nc.gpsimd.tensor_tensor(sM[0:64, :], sM[0:64, :], rbcM,
                                            ALU.mult)
                elif MULT_ENGINE == "split":
                    nc.gpsimd.tensor_tensor(sP, sP, rbcP, ALU.mult)
                    nc.vector.tensor_tensor(sM[0:64, :], sM[0:64, :], rbcM,
                                            ALU.mult)
                else:
                    nc.vector.tensor_tensor(sP, sP, rbcP, ALU.mult)
                    nc.vector.tensor_tensor(sM[0:64, :], sM[0:64, :], rbcM,
                                            ALU.mult)

                nc.sync.dma_start(out=vA[st], in_=sA)
                nc.sync.dma_start(out=vB[st], in_=sB)
                nc.sync.dma_start(out=vP[st], in_=sP)
                nc.sync.dma_start(out=vMpi[st], in_=sM[0:64, :])
                nc.sync.dma_start(out=vMsg[st], in_=sM[64:128, :])
                nc.sync.dma_start(out=vMb[st], in_=sMb)

    # All ACT functions used (Exp, Ln, Copy) live in one table set; restrict
    # the chooser to it so the kernel performs a single ACT_TABLE_LOAD
    # instead of thrashing between exp/ln sets (~2.7us per reload).
    import concourse.bacc as bacc_mod
    orig_tables = bacc_mod.get_activation_tables
    def _only_nle(arch):
        t = orig_tables(arch)
        name = "natural_log_exp_and_others"
        if name not in t:
            return t
        return {k: (v if k == name else set()) for k, v in t.items()}
    bacc_mod.get_activation_tables = _only_nle
    try:
        nc.compile()
    finally:
        bacc_mod.get_activation_tables = orig_tables
    return nc


def _prep_weights(i):
    f = np.float32
    lw1 = np.zeros((24, 128), f)
    for g in range(G):
        lw1[6 * g:6 * g + 5, 32 * g:32 * (g + 1)] = i["w1"].T
        lw1[6 * g + 5, 32 * g:32 * (g + 1)] = i["b1"]
    lw2 = np.zeros((128, 128), f)
    lw3 = np.zeros((128, 128), f)
    for g in range(G):
        lw2[32 * g:32 * (g + 1), 32 * g:32 * (g + 1)] = i["w2"].T
        lw3[32 * g:32 * (g + 1), 32 * g:32 * (g + 1)] = i["w3"].T

    def pair_chunk(w0, w1):
        # g-major pair: out col = g*32 + h*16 + k
        l = np.zeros((128, 128), f)
        for g in range(G):
            l[32 * g:32 * (g + 1), 32 * g:32 * g + 16] = w0.T
            l[32 * g:32 * (g + 1), 32 * g + 16:32 * g + 32] = w1.T
        return l

    def half_chunks(w0, w1=None):
        # h-major: cols 0:64 head0 (g-major k), cols 64:128 head1
        ncol = 64 if w1 is None else 128
        l = np.zeros((128, ncol), f)
        for g in range(G):
            l[32 * g:32 * (g + 1), 16 * g:16 * (g + 1)] = w0.T
            if w1 is not None:
                l[32 * g:32 * (g + 1), 64 + 16 * g:64 + 16 * (g + 1)] = w1.T
        return l

    lA = pair_chunk(i["rmu_w"], i["gmu_w"])
    lB = pair_chunk(i["rsg_w"], i["gsg_w"])
    lP = pair_chunk(i["rpi_w"], i["gpi_w"])
    lM = half_chunks(i["bpi_w"], i["bsg_w"])
    lE = half_chunks(i["bmu_w"])

    lrA = np.zeros((24, 128), f)
    lrE = np.zeros((24, 64), f)
    for g in range(G):
        for k in range(K):
            lrA[6 * g + 0, 32 * g + k] = 1.0           # + x_r for mu_r
            lrA[6 * g + 5, 32 * g + k] = i["rmu_b"][k]
            lrA[6 * g + 1, 32 * g + 16 + k] = 1.0      # + x_g for mu_g
            lrA[6 * g + 5, 32 * g + 16 + k] = i["gmu_b"][k]
            lrE[6 * g + 2, 16 * g + k] = 1.0           # + x_b for mu_b
            lrE[6 * g + 5, 16 * g + k] = i["bmu_b"][k]

    # fused softmax sum+broadcast: 16x16 ones blocks on the diagonal
    blk = np.ones((16, 16), f)
    lsPbig = np.kron(np.eye(8, dtype=f), blk)          # [128, 128]
    lsMbig = np.kron(np.eye(4, dtype=f), blk)          # [64, 64]

    col = lambda v: np.ascontiguousarray(v.reshape(-1, 1).astype(f))

    def pair_bias(b0, b1):
        v = np.zeros(128, f)
        for g in range(G):
            v[32 * g:32 * g + 16] = b0
            v[32 * g + 16:32 * g + 32] = b1
        return col(v)

    bb2 = col(np.tile(i["b2"], G))
    bb3 = col(np.tile(i["b3"], G))
    bB = pair_bias(i["rsg_b"], i["gsg_b"])
    bP = pair_bias(i["rpi_b"], i["gpi_b"])
    bM = col(np.concatenate([np.tile(i["bpi_b"], G), np.tile(i["bsg_b"], G)]))

    w = {"lw1": lw1, "lw2": lw2, "lw3": lw3, "lA": lA, "lB": lB, "lP": lP,
         "lM": lM, "lE": lE, "lrA": lrA, "lrE": lrE,
         "lsPbig": lsPbig, "lsMbig": lsMbig}
    w = {k: v.astype(NPBF) for k, v in w.items()}
    w.update({"bb2": bb2, "bb3": bb3, "bB": bB, "bP": bP, "bM": bM})
    return w


def _get_runner():
    """Compile the Bass program once and wrap it in a cached sharded jit."""
    if "runner" in _CACHE:
        return _CACHE["runner"]
    import jax
    from jax.sharding import Mesh, PartitionSpec
    from jax.experimental.shard_map import shard_map
    import concourse.mybir as mb
    import concourse.bass2jax as b2j

    nc = _CACHE.get("nc")
    if nc is None:
        nc = _CACHE["nc"] = _build_program()

    b2j.install_neuronx_cc_hook()
    partition_name = (nc.partition_id_tensor.name
                      if nc.partition_id_tensor else None)
    in_names, out_names, out_avals = [], [], []
    for alloc in nc.m.functions[0].allocations:
        if not isinstance(alloc, mb.MemoryLocationSet):
            continue
        name = alloc.memorylocations[0].name
        if alloc.kind == "ExternalInput":
            if name != partition_name:
                in_names.append(name)
        elif alloc.kind == "ExternalOutput":
            out_names.append(name)
            out_avals.append(jax.core.ShapedArray(
                tuple(alloc.tensor_shape), mb.dt.np(alloc.dtype)))
    n_params = len(in_names)
    bind_names = list(in_names + out_names)
    if partition_name is not None:
        bind_names.append(partition_name)
    bind_names = tuple(bind_names)

    def _body(*args):
        operands = list(args)
        if partition_name is not None:
            operands.append(b2j.partition_id_tensor())
        outs = b2j._bass_exec_p.bind(
            *operands,
            out_avals=tuple(out_avals),
            in_names=bind_names,
            out_names=tuple(out_names),
            lowering_input_output_aliases=(),
            sim_require_finite=True,
            sim_require_nnan=True,
            nc=nc,
        )
        return tuple(outs)

    devices = jax.devices()[:NCORES]
    mesh = Mesh(np.asarray(devices), ("core",))
    nin = n_params + len(out_names)
    fn = jax.jit(
        shard_map(_body, mesh=mesh,
                  in_specs=(PartitionSpec("core"),) * nin,
                  out_specs=(PartitionSpec("core"),) * len(out_names),
                  check_rep=False),
        keep_unused=True,
    )
    zeros = [np.zeros((NCORES * a.shape[0], *a.shape[1:]), a.dtype)
             for a in out_avals]
    runner = {"fn": fn, "in_names": in_names, "out_names": out_names,
              "out_avals": out_avals, "zeros": zeros, "mesh": mesh}
    _CACHE["runner"] = runner
    return runner


def _make_concat_inputs(inputs):
    wmaps = _prep_weights(inputs)
    x = inputs["x"]  # [B, 5, H, W]
    xs = []
    for c in range(NCORES):
        xc = x[:, :, c * HC:(c + 1) * HC, :].reshape(B, CIN, PXB)
        xa = np.empty((B, CIN + 1, PXB), np.float32)
        xa[:, :CIN] = xc
        xa[:, CIN] = 1.0
        xs.append(xa.astype(NPBF))
    per_core = {"xin": np.concatenate(xs, axis=0)}
    for n, w in wmaps.items():
        per_core[n] = np.concatenate([w] * NCORES, axis=0)
    return per_core


def kernel(**inputs):
    inputs = {k: np.asarray(v, dtype=np.float32) for k, v in inputs.items()}
    runner = _get_runner()
    concat = _make_concat_inputs(inputs)
    args = [concat[n] for n in runner["in_names"]]
    outs = runner["fn"](*args, *runner["zeros"])
    res = {}
    for name, aval, arr in zip(runner["out_names"], runner["out_avals"], outs):
        res[name] = np.asarray(arr).reshape(NCORES, *aval.shape)

    def gather(name, ch0):
        parts = [res[name][c][:, ch0:ch0 + K, :].astype(np.float32)
                 .reshape(B, K, HC, W) for c in range(NCORES)]
        return np.concatenate(parts, axis=2)

    mu_r, mu_g = gather("oA", 0), gather("oA", K)
    sg_r, sg_g = gather("oB", 0), gather("oB", K)
    pi_r, pi_g = gather("oP", 0), gather("oP", K)
    pi_b, sg_b = gather("oM", 0), gather("oM", K)

    # oMb is column-folded: partition = 64*f + 16*g + k over (s g f n)
    mb_parts = []
    for c in range(NCORES):
        a = res["oMb"][c].astype(np.float32)  # [B, K, PXB]
        mb_parts.append(a.reshape(B, K, HC, W))
    mu_b = np.concatenate(mb_parts, axis=2)
    return (mu_r, sg_r, pi_r, mu_g, sg_g, pi_g, mu_b, sg_b, pi_b)


# revision 14
# speedup vs baseline: 200.1536x; 1.1162x over previous
"""Trainium2 Bass kernel for per-pixel MDN head (nn_MDN_38946763440904).

Reference computation (per pixel, channels-first):
  h      = relu(W1 @ x5 + b1)        # 5  -> 32
  h      = relu(W2 @ h + b2)         # 32 -> 32
  latent = relu(W3 @ h + b3)         # 32 -> 32
  for c in (r, g, b):
      mu_c    = Wmu_c @ latent + bmu_c + x[c]
      sigma_c = softplus(Wsg_c @ latent + bsg_c)
      pi_c    = softmax(Wpi_c @ latent + bpi_c)   # over the 16 components

Strategy: shard H across the 8 cores (each core gets [4, 5, 64, 512]).
On-core, pixels are processed in supertiles of 4 groups x 2048 pixels;
each group's 32 latent channels occupy 32 SBUF partitions, so all
matmuls are dense 128-partition block-diagonal matmuls.  All matmul
operands and all DRAM outputs are bf16 (PSUM accumulation stays fp32);
this halves the HBM write traffic (the memory roofline for this
problem) and enables the PE fast-weight-load path.

Head outputs are packed into chunks so that every chunk needs one
uniform elementwise treatment:
  A  = [mu_r | mu_g] (g-major, 128 rows)     -> copy
  Bc = [sg_r | sg_g] (g-major)               -> exp, ln(1+e)
  P  = [pi_r | pi_g] (g-major)               -> exp, softmax-normalize
  M  = [pi_b ; sg_b] (64+64 rows)            -> exp, then ln on sg half
  Mb = [mu_b] column-folded to [128, 1024]   -> copy
mu residual+bias ride the head matmuls via an augmented-x input
(5 channels + ones row).  The softmax normalizer uses a fused
sum+broadcast matmul (block-diagonal ones weights produce the 16-wide
block sums already replicated to all 16 rows), a fast DVE reciprocal
(bit-trick + Newton, ~51 ULP), and one multiply.  Exp/Ln live in one
ACT table set (natural_log_exp_and_others) so the table loads once.
"""

import os
import sys

if "/opt/trn_rl_repo" not in sys.path:
    sys.path.insert(0, "/opt/trn_rl_repo")

import numpy as np
import ml_dtypes

import concourse.bass as bass
import concourse.mybir as mybir
import concourse.tile as tile
from concourse import bacc
from concourse.bass_utils import run_bass_kernel_spmd

F32 = mybir.dt.float32
BF16 = mybir.dt.bfloat16
AF = mybir.ActivationFunctionType
ALU = mybir.AluOpType

B, CIN, H, W = 4, 5, 512, 512
K, LAT = 16, 32
NCORES = 8
HC = H // NCORES            # 64 rows of H per core
PXB = HC * W                # 32768 pixels per batch image per core
G = 4                       # pixel groups per supertile
COLS = 2048                 # pixels per group per supertile
ST_PER_B = PXB // (G * COLS)  # supertiles per batch image (4)

NPBF = ml_dtypes.bfloat16

_CACHE = {}

# engine-assignment knobs (tuned from traces)
MULT_ENGINE = os.environ.get("MDN_MULT", "split")   # gps | dve | split
COPY_ENGINE = os.environ.get("MDN_COPY", "act")     # act | dve


def _build_program(variant="full"):
    nc = bacc.Bacc("TRN2", target_bir_lowering=False, debug=False)

    xin = nc.dram_tensor("xin", [B, CIN + 1, PXB], BF16, kind="ExternalInput")

    wnames_b = {
        "lw1": [24, 128], "lw2": [128, 128], "lw3": [128, 128],
        "lA": [128, 128], "lB": [128, 128], "lP": [128, 128],
        "lM": [128, 128], "lE": [128, 64],
        "lrA": [24, 128], "lrE": [24, 64],
        "lsPbig": [128, 128], "lsMbig": [64, 64],
    }
    wnames_f = {
        "bb2": [128, 1], "bb3": [128, 1],
        "bB": [128, 1], "bP": [128, 1], "bM": [128, 1],
    }
    dram_w = {}
    for n, shp in wnames_b.items():
        dram_w[n] = nc.dram_tensor(n, shp, BF16, kind="ExternalInput")
    for n, shp in wnames_f.items():
        dram_w[n] = nc.dram_tensor(n, shp, F32, kind="ExternalInput")

    # paired outputs: channel = h*16 + k for the two heads in the chunk
    oA = nc.dram_tensor("oA", [B, 2 * K, PXB], BF16, kind="ExternalOutput")
    oB = nc.dram_tensor("oB", [B, 2 * K, PXB], BF16, kind="ExternalOutput")
    oP = nc.dram_tensor("oP", [B, 2 * K, PXB], BF16, kind="ExternalOutput")
    oM = nc.dram_tensor("oM", [B, 2 * K, PXB], BF16, kind="ExternalOutput")
    oMb = nc.dram_tensor("oMb", [B, K, PXB], BF16, kind="ExternalOutput")

    from contextlib import ExitStack
    with tile.TileContext(nc) as tc, ExitStack() as es:
        consts = es.enter_context(tc.tile_pool(name="consts", bufs=1))
        xpool = es.enter_context(tc.tile_pool(name="xp", bufs=2))
        hpool = es.enter_context(tc.tile_pool(name="hp", bufs=2))
        spool = es.enter_context(tc.tile_pool(name="sp", bufs=2))
        rpool = es.enter_context(tc.tile_pool(name="rp", bufs=2))
        ps_z = es.enter_context(tc.tile_pool(name="psz", bufs=2, space="PSUM"))
        ps_bc = es.enter_context(tc.tile_pool(name="psbc", bufs=2,
                                              space="PSUM"))

        # --- load constants once ---
        wt = {}
        for n, shp in {**wnames_b, **wnames_f}.items():
            dt = BF16 if n in wnames_b else F32
            t = consts.tile(shp, dt, tag=n, name=n)
            nc.sync.dma_start(out=t, in_=dram_w[n][:, :])
            wt[n] = t

        # DRAM views: [st] -> [g, hk, n] (g-major paired chunks)
        def view2(o, b_):
            return o[b_, :, :].rearrange("hk (s g n) -> s g hk n",
                                         s=ST_PER_B, g=G, n=COLS)

        def view1(o, b_, ch0=0):
            return o[b_, ch0:ch0 + K, :].rearrange("k (s g n) -> s g k n",
                                                   s=ST_PER_B, g=G, n=COLS)

        for b_ in range(B):
            vA, vB, vP = view2(oA, b_), view2(oB, b_), view2(oP, b_)
            vMpi, vMsg = view1(oM, b_, 0), view1(oM, b_, K)
            vMb = oMb[b_, :, :].rearrange("k (s g n) -> s g k n",
                                          s=ST_PER_B, g=G, n=COLS)
            # prefetch all x tiles for this image on the ACT HWDGE queue so
            # input loads never queue behind the output stores on Sync
            x_ts = []
            for st in range(ST_PER_B):
                base = st * G * COLS
                x_t = xpool.tile([24, COLS], BF16, tag="x", name="x_t",
                                 bufs=ST_PER_B + 1)
                nc.scalar.dma_start(
                    out=x_t,
                    in_=xin[b_, :, base:base + G * COLS].rearrange(
                        "c (g n) -> g c n", n=COLS),
                )
                x_ts.append(x_t)
            for st in range(ST_PER_B):
                x_t = x_ts[st]

                sA = spool.tile([128, COLS], BF16, tag="sA", name="sA")
                sB = spool.tile([128, COLS], BF16, tag="sB", name="sB")
                sP = spool.tile([128, COLS], BF16, tag="sP", name="sP")
                sM = spool.tile([128, COLS], BF16, tag="sM", name="sM")
                sMb = spool.tile([64, COLS], BF16, tag="sMb", name="sMb")

                def layer(dst, wname, src, bias=None):
                    """backbone layer: 2 half-tiles of [128,1024] psum,
                    relu(+bias) evacuated on DVE into dst (bf16)."""
                    for hh in range(2):
                        z = ps_z.tile([128, 1024], F32, tag="z", name="z")
                        for q in range(2):
                            cs = slice(hh * 1024 + q * 512,
                                       hh * 1024 + q * 512 + 512)
                            zs = slice(q * 512, q * 512 + 512)
                            nc.tensor.matmul(z[:, zs], wt[wname], src[:, cs],
                                             start=True, stop=True)
                        dsl = dst[:, hh * 1024:(hh + 1) * 1024]
                        if bias is None:
                            nc.vector.tensor_scalar(dsl, z, 0.0, None,
                                                    ALU.max)
                        else:
                            nc.vector.tensor_scalar(dsl, z, wt[bias], 0.0,
                                                    ALU.add, ALU.max)

                # --- backbone ---
                h1 = hpool.tile([128, COLS], BF16, tag="h1", name="h1")
                layer(h1, "lw1", x_t)
                h2 = hpool.tile([128, COLS], BF16, tag="h2", name="h2")
                layer(h2, "lw2", h1, bias="bb2")
                lat = hpool.tile([128, COLS], BF16, tag="lat", name="lat")
                layer(lat, "lw3", h2, bias="bb3")

                copy_eng = nc.scalar if COPY_ENGINE == "act" else nc.vector

                # --- mu chunk A = [mu_r | mu_g] (residual via lrA) ---
                for hh in range(2):
                    z = ps_z.tile([128, 1024], F32, tag="z", name="z")
                    for q in range(2):
                        cs = slice(hh * 1024 + q * 512,
                                   hh * 1024 + q * 512 + 512)
                        zs = slice(q * 512, q * 512 + 512)
                        nc.tensor.matmul(z[:, zs], wt["lA"], lat[:, cs],
                                         start=True, stop=False)
                    for q in range(2):
                        cs = slice(hh * 1024 + q * 512,
                                   hh * 1024 + q * 512 + 512)
                        zs = slice(q * 512, q * 512 + 512)
                        nc.tensor.matmul(z[:, zs], wt["lrA"], x_t[:, cs],
                                         start=False, stop=True)
                    if COPY_ENGINE == "act":
                        nc.scalar.copy(sA[:, hh * 1024:(hh + 1) * 1024], z)
                    else:
                        nc.vector.tensor_copy(
                            sA[:, hh * 1024:(hh + 1) * 1024], z)

                # --- mu_b chunk [64, 2048] (residual via lrE) ---
                for hh in range(2):
                    zmb = ps_z.tile([128, 1024], F32, tag="z", name="zmb")
                    for q in range(2):
                        cs = slice(hh * 1024 + q * 512,
                                   hh * 1024 + q * 512 + 512)
                        zs = slice(q * 512, q * 512 + 512)
                        nc.tensor.matmul(zmb[0:64, zs], wt["lE"],
                                         lat[:, cs], start=True, stop=False)
                    for q in range(2):
                        cs = slice(hh * 1024 + q * 512,
                                   hh * 1024 + q * 512 + 512)
                        zs = slice(q * 512, q * 512 + 512)
                        nc.tensor.matmul(zmb[0:64, zs], wt["lrE"],
                                         x_t[:, cs], start=False, stop=True)
                    dsl = sMb[:, hh * 1024:(hh + 1) * 1024]
                    if COPY_ENGINE == "act":
                        nc.scalar.copy(dsl, zmb[0:64, :])
                    else:
                        nc.vector.tensor_copy(dsl, zmb[0:64, :])

                # --- sg/pi chunks: z then exp (bias pre-exp) ---
                for name, stile, btile in (("lB", sB, "bB"), ("lP", sP, "bP"),
                                           ("lM", sM, "bM")):
                    for hh in range(2):
                        z = ps_z.tile([128, 1024], F32, tag="z", name="z")
                        for q in range(2):
                            cs = slice(hh * 1024 + q * 512,
                                       hh * 1024 + q * 512 + 512)
                            zs = slice(q * 512, q * 512 + 512)
                            nc.tensor.matmul(z[:, zs], wt[name], lat[:, cs],
                                             start=True, stop=True)
                        nc.scalar.activation(
                            stile[:, hh * 1024:(hh + 1) * 1024], z, AF.Exp,
                            bias=wt[btile])

                # softplus finalize: ln(e + 1)
                nc.scalar.activation(sB, sB, AF.Ln, bias=1.0)
                nc.scalar.activation(sM[64:128, :], sM[64:128, :], AF.Ln,
                                     bias=1.0)

                # --- softmax normalize: fused sum+broadcast matmul,
                #     fast reciprocal, multiply ---
                rbcP = rpool.tile([128, COLS], F32, tag="rbcP", name="rbcP")
                rbcM = rpool.tile([64, COLS], F32, tag="rbcM", name="rbcM")
                for hh in range(2):
                    bcp = ps_bc.tile([128, 1024], F32, tag="bc", name="bcp")
                    for q in range(2):
                        cs = slice(hh * 1024 + q * 512,
                                   hh * 1024 + q * 512 + 512)
                        zs = slice(q * 512, q * 512 + 512)
                        nc.tensor.matmul(bcp[:, zs], wt["lsPbig"], sP[:, cs],
                                         start=True, stop=True)
                    nc.vector.reciprocal_approx_fast(
                        rbcP[:, hh * 1024:(hh + 1) * 1024], bcp)
                    bcm = ps_bc.tile([128, 1024], F32, tag="bc", name="bcm")
                    for q in range(2):
                        cs = slice(hh * 1024 + q * 512,
                                   hh * 1024 + q * 512 + 512)
                        zs = slice(q * 512, q * 512 + 512)
                        nc.tensor.matmul(bcm[0:64, zs], wt["lsMbig"],
                                         sM[0:64, cs], start=True, stop=True)
                    nc.vector.reciprocal_approx_fast(
                        rbcM[:, hh * 1024:(hh + 1) * 1024], bcm[0:64, :])

                # separate output tiles for the normalized pi so the
                # multiply doesn't create in-place WAR chains on sP/sM
                oPt = spool.tile([128, COLS], BF16, tag="oPt", name="oPt")
                oMt = spool.tile([64, COLS], BF16, tag="oMt", name="oMt")
                if MULT_ENGINE == "gps":
                    nc.gpsimd.tensor_tensor(oPt, sP, rbcP, ALU.mult)
                    nc.gpsimd.tensor_tensor(oMt, sM[0:64, :], rbcM, ALU.mult)
                elif MULT_ENGINE == "split":
                    nc.gpsimd.tensor_tensor(oPt, sP, rbcP, ALU.mult)
                    nc.vector.tensor_tensor(oMt, sM[0:64, :], rbcM, ALU.mult)
                else:
                    nc.vector.tensor_tensor(oPt, sP, rbcP, ALU.mult)
                    nc.vector.tensor_tensor(oMt, sM[0:64, :], rbcM, ALU.mult)

                nc.sync.dma_start(out=vA[st], in_=sA)
                nc.sync.dma_start(out=vB[st], in_=sB)
                nc.sync.dma_start(out=vP[st], in_=oPt)
                nc.sync.dma_start(out=vMpi[st], in_=oMt)
                nc.sync.dma_start(out=vMsg[st], in_=sM[64:128, :])
                nc.sync.dma_start(out=vMb[st], in_=sMb)

    # All ACT functions used (Exp, Ln, Copy) live in one table set; restrict
    # the chooser to it so the kernel performs a single ACT_TABLE_LOAD
    # instead of thrashing between exp/ln sets (~2.7us per reload).
    import concourse.bacc as bacc_mod
    orig_tables = bacc_mod.get_activation_tables
    def _only_nle(arch):
        t = orig_tables(arch)
        name = "natural_log_exp_and_others"
        if name not in t:
            return t
        return {k: (v if k == name else set()) for k, v in t.items()}
    bacc_mod.get_activation_tables = _only_nle
    try:
        nc.compile()
    finally:
        bacc_mod.get_activation_tables = orig_tables
    return nc


def _prep_weights(i):
    f = np.float32
    lw1 = np.zeros((24, 128), f)
    for g in range(G):
        lw1[6 * g:6 * g + 5, 32 * g:32 * (g + 1)] = i["w1"].T
        lw1[6 * g + 5, 32 * g:32 * (g + 1)] = i["b1"]
    lw2 = np.zeros((128, 128), f)
    lw3 = np.zeros((128, 128), f)
    for g in range(G):
        lw2[32 * g:32 * (g + 1), 32 * g:32 * (g + 1)] = i["w2"].T
        lw3[32 * g:32 * (g + 1), 32 * g:32 * (g + 1)] = i["w3"].T

    def pair_chunk(w0, w1):
        # g-major pair: out col = g*32 + h*16 + k
        l = np.zeros((128, 128), f)
        for g in range(G):
            l[32 * g:32 * (g + 1), 32 * g:32 * g + 16] = w0.T
            l[32 * g:32 * (g + 1), 32 * g + 16:32 * g + 32] = w1.T
        return l

    def half_chunks(w0, w1=None):
        # h-major: cols 0:64 head0 (g-major k), cols 64:128 head1
        ncol = 64 if w1 is None else 128
        l = np.zeros((128, ncol), f)
        for g in range(G):
            l[32 * g:32 * (g + 1), 16 * g:16 * (g + 1)] = w0.T
            if w1 is not None:
                l[32 * g:32 * (g + 1), 64 + 16 * g:64 + 16 * (g + 1)] = w1.T
        return l

    lA = pair_chunk(i["rmu_w"], i["gmu_w"])
    lB = pair_chunk(i["rsg_w"], i["gsg_w"])
    lP = pair_chunk(i["rpi_w"], i["gpi_w"])
    lM = half_chunks(i["bpi_w"], i["bsg_w"])
    lE = half_chunks(i["bmu_w"])

    lrA = np.zeros((24, 128), f)
    lrE = np.zeros((24, 64), f)
    for g in range(G):
        for k in range(K):
            lrA[6 * g + 0, 32 * g + k] = 1.0           # + x_r for mu_r
            lrA[6 * g + 5, 32 * g + k] = i["rmu_b"][k]
            lrA[6 * g + 1, 32 * g + 16 + k] = 1.0      # + x_g for mu_g
            lrA[6 * g + 5, 32 * g + 16 + k] = i["gmu_b"][k]
            lrE[6 * g + 2, 16 * g + k] = 1.0           # + x_b for mu_b
            lrE[6 * g + 5, 16 * g + k] = i["bmu_b"][k]

    # fused softmax sum+broadcast: 16x16 ones blocks on the diagonal
    blk = np.ones((16, 16), f)
    lsPbig = np.kron(np.eye(8, dtype=f), blk)          # [128, 128]
    lsMbig = np.kron(np.eye(4, dtype=f), blk)          # [64, 64]

    col = lambda v: np.ascontiguousarray(v.reshape(-1, 1).astype(f))

    def pair_bias(b0, b1):
        v = np.zeros(128, f)
        for g in range(G):
            v[32 * g:32 * g + 16] = b0
            v[32 * g + 16:32 * g + 32] = b1
        return col(v)

    bb2 = col(np.tile(i["b2"], G))
    bb3 = col(np.tile(i["b3"], G))
    bB = pair_bias(i["rsg_b"], i["gsg_b"])
    bP = pair_bias(i["rpi_b"], i["gpi_b"])
    bM = col(np.concatenate([np.tile(i["bpi_b"], G), np.tile(i["bsg_b"], G)]))

    w = {"lw1": lw1, "lw2": lw2, "lw3": lw3, "lA": lA, "lB": lB, "lP": lP,
         "lM": lM, "lE": lE, "lrA": lrA, "lrE": lrE,
         "lsPbig": lsPbig, "lsMbig": lsMbig}
    w = {k: v.astype(NPBF) for k, v in w.items()}
    w.update({"bb2": bb2, "bb3": bb3, "bB": bB, "bP": bP, "bM": bM})
    return w


def _get_runner():
    """Compile the Bass program once and wrap it in a cached sharded jit."""
    if "runner" in _CACHE:
        return _CACHE["runner"]
    import jax
    from jax.sharding import Mesh, PartitionSpec
    from jax.experimental.shard_map import shard_map
    import concourse.mybir as mb
    import concourse.bass2jax as b2j

    nc = _CACHE.get("nc")
    if nc is None:
        nc = _CACHE["nc"] = _build_program()

    b2j.install_neuronx_cc_hook()
    partition_name = (nc.partition_id_tensor.name
                      if nc.partition_id_tensor else None)
    in_names, out_names, out_avals = [], [], []
    for alloc in nc.m.functions[0].allocations:
        if not isinstance(alloc, mb.MemoryLocationSet):
            continue
        name = alloc.memorylocations[0].name
        if alloc.kind == "ExternalInput":
            if name != partition_name:
                in_names.append(name)
        elif alloc.kind == "ExternalOutput":
            out_names.append(name)
            out_avals.append(jax.core.ShapedArray(
                tuple(alloc.tensor_shape), mb.dt.np(alloc.dtype)))
    n_params = len(in_names)
    bind_names = list(in_names + out_names)
    if partition_name is not None:
        bind_names.append(partition_name)
    bind_names = tuple(bind_names)

    def _body(*args):
        operands = list(args)
        if partition_name is not None:
            operands.append(b2j.partition_id_tensor())
        outs = b2j._bass_exec_p.bind(
            *operands,
            out_avals=tuple(out_avals),
            in_names=bind_names,
            out_names=tuple(out_names),
            lowering_input_output_aliases=(),
            sim_require_finite=True,
            sim_require_nnan=True,
            nc=nc,
        )
        return tuple(outs)

    devices = jax.devices()[:NCORES]
    mesh = Mesh(np.asarray(devices), ("core",))
    nin = n_params + len(out_names)
    fn = jax.jit(
        shard_map(_body, mesh=mesh,
                  in_specs=(PartitionSpec("core"),) * nin,
                  out_specs=(PartitionSpec("core"),) * len(out_names),
                  check_rep=False),
        keep_unused=True,
    )
    zeros = [np.zeros((NCORES * a.shape[0], *a.shape[1:]), a.dtype)
             for a in out_avals]
    runner = {"fn": fn, "in_names": in_names, "out_names": out_names,
              "out_avals": out_avals, "zeros": zeros, "mesh": mesh}
    _CACHE["runner"] = runner
    return runner


def _make_concat_inputs(inputs):
    wmaps = _prep_weights(inputs)
    x = inputs["x"]  # [B, 5, H, W]
    xs = []
    for c in range(NCORES):
        xc = x[:, :, c * HC:(c + 1) * HC, :].reshape(B, CIN, PXB)
        xa = np.empty((B, CIN + 1, PXB), np.float32)
        xa[:, :CIN] = xc
        xa[:, CIN] = 1.0
        xs.append(xa.astype(NPBF))
    per_core = {"xin": np.concatenate(xs, axis=0)}
    for n, w in wmaps.items():
        per_core[n] = np.concatenate([w] * NCORES, axis=0)
    return per_core


def kernel(**inputs):
    inputs = {k: np.asarray(v, dtype=np.float32) for k, v in inputs.items()}
    runner = _get_runner()
    concat = _make_concat_inputs(inputs)
    args = [concat[n] for n in runner["in_names"]]
    outs = runner["fn"](*args, *runner["zeros"])
    res = {}
    for name, aval, arr in zip(runner["out_names"], runner["out_avals"], outs):
        res[name] = np.asarray(arr).reshape(NCORES, *aval.shape)

    def gather(name, ch0):
        parts = [res[name][c][:, ch0:ch0 + K, :].astype(np.float32)
                 .reshape(B, K, HC, W) for c in range(NCORES)]
        return np.concatenate(parts, axis=2)

    mu_r, mu_g = gather("oA", 0), gather("oA", K)
    sg_r, sg_g = gather("oB", 0), gather("oB", K)
    pi_r, pi_g = gather("oP", 0), gather("oP", K)
    pi_b, sg_b = gather("oM", 0), gather("oM", K)

    # oMb is column-folded: partition = 64*f + 16*g + k over (s g f n)
    mb_parts = []
    for c in range(NCORES):
        a = res["oMb"][c].astype(np.float32)  # [B, K, PXB]
        mb_parts.append(a.reshape(B, K, HC, W))
    mu_b = np.concatenate(mb_parts, axis=2)
    return (mu_r, sg_r, pi_r, mu_g, sg_g, pi_g, mu_b, sg_b, pi_b)
